# revision 24
# baseline (speedup 1.0000x reference)
"""Trainium2 Bass kernel for nn_AdaptiveGraphConvolutionalLSTM.

Reference computation (B=32, F=1024):
    gc_R  = concat_k( einsum('bf,bfg->bg', x, GC_Rk_w  * Rk_i) )   k=1..3
    gc_NR = concat_k( einsum('bf,bfg->bg', x, GC_NRk_w * Rk_i) )   (NR reuses R masks)
    combined = concat([gc_R, gc_NR, hidden])                        # [B, 7F]
    f,i,o = sigmoid(combined @ {fl,il,ol}_w.T + b); C = tanh(combined @ Cl_w.T + Cl_b)
    Cell = f*cell + i*C ; Hidden = o*tanh(Cell)

Distribution over 8 NeuronCores: GC output dim g and gate output columns
sharded (128 columns/core); NR*_i masks never read (unused by reference).

Core GC restructure vs the earlier version (which computed u = w*m on
VectorE twice per mask element, once per branch):
  - v = x (*) m is computed ONCE per mask element (VectorE TT, bf16 2x),
    shared by both branches.
  - PE stationary = "w-pair" tile [f-chunk, (br,g')] holding the R and NR
    GC weight columns side by side; moving = v. psum out[(br,g'),(g,b)]
    contains both branches' GC results on the g'==g diagonal, extracted
    with one DRAM bounce + strided-AP diagonal DMAs.
  - Masks k=1,2 are stored fp8e4m3 (halves their HBM bytes) and converted
    to bf16 on ScalarE/VectorE before the multiply; k=0 stays bf16.
Gate path (AllGather of combined^T chunks + fp8 gate matmuls + LSTM cell)
is unchanged in structure; gate weights are fp8 (x512 host pre-scale,
un-scaled inside sigmoid/tanh).

kernel(**inputs) takes the FULL inputs and returns (Hidden, Cell) full.
"""
import numpy as np
import ml_dtypes

from concourse import bass, bacc, tile, mybir
from concourse.bass_utils import run_bass_kernel_spmd

BF16 = ml_dtypes.bfloat16
FP8 = ml_dtypes.float8_e4m3
GW_SCALE = 512.0   # gate weights ~U(-0.012, 0.012); x512 puts them in
                   # e4m3's normal range; sigmoid/tanh un-scale by 1/512
B, F, K, NCORES = 32, 1024, 3, 8
P = 128          # partitions / f-chunk size
FC = F // P      # 8 f-chunks
G = F // NCORES  # 128 g-columns per core
GH = G // 2      # 64 g-half
HB = B // 2      # 16 half batch
GBB = G * B      # 4096, one fc-block of mask columns (g-major, b-minor)
NKC = 7 * FC + 1  # 57 gate contraction chunks (48 gathered + 8 hidden + bias)

# per-mask dtype: "bf" = bf16 direct, "f8" = fp8e4m3 + on-device convert
MASK_DT = ("bf", "f8", "f8")
# (k, fc) pairs whose fp8->bf16 convert runs on VectorE / GpSimd instead
# of ScalarE
DVE_CONV = {(2, 5), (2, 6), (2, 7)}
FUSE_F8 = False
GP_CONV = set()
# issue bounce/diag/gate-chunk/output DMAs from GpSimd's SWDGE instead of
# the sync-engine HWDGE ring (isolates them from bulk mask streaming)
GP_DMA = False

_DT_BF = mybir.dt.bfloat16
_DT_F32 = mybir.dt.float32
_DT_F8 = mybir.dt.float8e4

# bf16 const buffer layout: [wp_k0 | xT | hp | wp_k1 | wp_k2]
WPC = 2 * FC * P                 # 2048 cols per k of w-pair tiles
OFF_XT = WPC                     # 2048
OFF_HP = WPC + FC * B            # 2304
OFF_W1 = OFF_HP + 9 * B          # 2592
OFF_W2 = OFF_W1 + WPC            # 4640
NCONST = OFF_W2 + WPC            # 6688
WP_BASE = (0, OFF_W1, OFF_W2)


def build_nc(reps: int = 1):
    """Build the SPMD per-core program. reps>1 repeats the compute body
    back-to-back inside one NEFF (for timing); reps=1 is the real kernel."""
    nc = bacc.Bacc("TRN2", target_bir_lowering=False, debug=False,
                   num_devices=NCORES)

    p_m = [nc.dram_tensor(f"m{k}", [P, FC * GBB],
                          _DT_BF if MASK_DT[k] == "bf" else _DT_F8,
                          kind="ExternalInput") for k in range(K)]
    p_const = nc.dram_tensor("cst", [P, NCONST], _DT_BF,
                             kind="ExternalInput")
    p_gw = nc.dram_tensor("gw", [P, NKC * 4 * P], _DT_F8,
                          kind="ExternalInput")
    p_cs = nc.dram_tensor("cs", [B, G], _DT_F32, kind="ExternalInput")
    p_out = nc.dram_tensor("out", [2, B, G], _DT_F32, kind="ExternalOutput")

    AF = mybir.ActivationFunctionType
    QGBB = 2 * GBB   # mask quarter = 2 fc blocks
    with tile.TileContext(nc) as tc:
        with tc.tile_pool(name="mqb", bufs=3) as mqb, \
             tc.tile_pool(name="mq8", bufs=5) as mq8, \
             tc.tile_pool(name="m16", bufs=2) as m16p, \
             tc.tile_pool(name="vp", bufs=9) as vp, \
             tc.tile_pool(name="cst", bufs=1) as cst, \
             tc.tile_pool(name="stg", bufs=3) as stg, \
             tc.tile_pool(name="ctp", bufs=3) as ctp, \
             tc.tile_pool(name="sml", bufs=2) as sml, \
             tc.tile_pool(name="pgc", bufs=4, space="PSUM") as pgc, \
             tc.tile_pool(name="dsc", bufs=3, space="DRAM") as dsc, \
             tc.tile_pool(name="dcc", bufs=3, space="DRAM") as dcc, \
             tc.tile_pool(name="dgg", bufs=3, space="DRAM") as dgg:

            # loads needed first (k=0 compute): wp_k0 + xT + hp in one DMA
            const_t = cst.tile([P, NCONST], _DT_BF, tag="cst")
            nc.sync.dma_start(out=const_t[:, 0:OFF_W1],
                              in_=p_const[:, 0:OFF_W1])

            def load_mask(k):
                # 4 quarter tiles per mask so compute starts early and
                # SBUF holds at most ~1.5 masks at a time
                tiles = []
                for q in range(4):
                    if MASK_DT[k] == "bf":
                        mt = mqb.tile([P, QGBB], _DT_BF, tag="mqb",
                                      name="mqb")
                    else:
                        mt = mq8.tile([P, QGBB], _DT_F8, tag="mq8",
                                      name="mq8")
                    nc.sync.dma_start(
                        out=mt[:, :],
                        in_=p_m[k][:, q * QGBB:(q + 1) * QGBB])
                    tiles.append(mt)
                return tiles

            m_tiles0 = load_mask(0)

            # remaining resident loads (queue behind k=0 masks)
            nc.sync.dma_start(out=const_t[:, OFF_W1:],
                              in_=p_const[:, OFF_W1:])
            cs_t = cst.tile([B, G], _DT_F32, tag="cs")
            nc.sync.dma_start(out=cs_t[:, :], in_=p_cs[:, :])
            # gate weights packed (kc, gate, m): h+bias chunks 48..56 and
            # gathered chunks 0..47, all resident fp8
            gwh_t = cst.tile([P, 9 * 4 * P], _DT_F8, tag="gwh")
            nc.sync.dma_start(out=gwh_t[:, :], in_=p_gw[:, 48 * 4 * P:])
            gw_a = cst.tile([P, 32 * 4 * P], _DT_F8, tag="gwA")
            nc.sync.dma_start(out=gw_a[:, :], in_=p_gw[:, 0:32 * 4 * P])
            gw_b = cst.tile([P, 16 * 4 * P], _DT_F8, tag="gwB")
            nc.sync.dma_start(out=gw_b[:, :],
                              in_=p_gw[:, 32 * 4 * P:48 * 4 * P])

            def wp_ap(k, gh, fc):
                off = WP_BASE[k] + (gh * FC + fc) * P
                return const_t[:, off:off + P]

            for rep in range(reps):
                m_tiles = m_tiles0 if rep == 0 else load_mask(0)

                # ---- phase A: hidden-state + bias gate partials ----
                # own short-lived psum group; gates accumulate in SBUF f32
                # (gacc) so all 8 PSUM banks are free for the GC tiles
                pgA = pgc.tile([B, 4 * G], _DT_F32, tag="gc", name="pgA")
                for kc in range(48, 57):
                    lhs = const_t[:, OFF_HP + (kc - 48) * B:
                                  OFF_HP + (kc - 47) * B]
                    nc.tensor.matmul(
                        pgA[:, :], lhsT=lhs,
                        rhs=gwh_t[:, (kc - 48) * 4 * P:(kc - 47) * 4 * P],
                        start=(kc == 48), stop=(kc == 56))
                gacc = sml.tile([B, 4 * G], _DT_F32, tag="gacc", name="gacc")
                nc.vector.tensor_copy(gacc[:, :], pgA[:, :])

                # contrib tensors are [feature, b] (transposed) so the
                # diagonal DMA has contiguous b innermost on both sides,
                # and gathered chunks load straight into lhsT layout
                contribA = dcc.tile([4 * G, B], _DT_BF, tag="cA", name="cA")
                contribB = dcc.tile([2 * G, B], _DT_BF, tag="cB", name="cB")
                gathA = gathB = None

                for k in range(K):
                    # ---- v = x (*) mask, per (fc, gh) tile ----
                    vt = {}
                    for fc in range(FC):
                        msrc = m_tiles[fc // 2][:, (fc % 2) * GBB:
                                                (fc % 2 + 1) * GBB]
                        if MASK_DT[k] == "bf" or FUSE_F8:
                            m16_3 = msrc.rearrange("p (g b) -> p g b", b=B)
                        else:
                            m16 = m16p.tile([P, GBB], _DT_BF, tag="m16",
                                            name="m16")
                            if (k, fc) in GP_CONV:
                                nc.gpsimd.tensor_copy(m16[:, :], msrc)
                            elif (k, fc) in DVE_CONV:
                                nc.vector.tensor_copy(m16[:, :], msrc)
                            else:
                                nc.scalar.activation(m16[:, :], msrc,
                                                     AF.Copy)
                            m16_3 = m16[:, :].rearrange("p (g b) -> p g b",
                                                        b=B)
                        x_ap = (const_t[:, OFF_XT + fc * B:
                                        OFF_XT + (fc + 1) * B]
                                .unsqueeze(1).broadcast_to([P, GH, B]))
                        for gh in range(2):
                            v = vp.tile([P, GH * B], _DT_BF, tag="v",
                                        name="v")
                            nc.vector.tensor_mul(
                                v[:, :].rearrange("p (g b) -> p g b", b=B),
                                m16_3[:, gh * GH:(gh + 1) * GH, :], x_ap)
                            vt[(fc, gh)] = v

                    # ---- GC matmuls + per-gh extraction ----
                    for gh in range(2):
                        pss = {}
                        for bh in range(2):
                            pss[(gh, bh)] = pgc.tile(
                                [P, 2 * 512], _DT_F32, tag="gc", name="gcps")
                        for fc in range(FC):
                            lhsT = wp_ap(k, gh, fc)
                            v3 = vt[(fc, gh)][:, :].rearrange(
                                "p (g b) -> p g b", b=B)
                            for bh in range(2):
                                for bq in range(2):
                                    b0 = bh * HB + bq * 8
                                    nc.tensor.matmul(
                                        pss[(gh, bh)][:, bq * 512:
                                                      (bq + 1) * 512],
                                        lhsT=lhsT,
                                        rhs=v3[:, :, b0:b0 + 8],
                                        start=(fc == 0),
                                        stop=(fc == FC - 1))
                        for (gh2, bh) in list(pss):
                            stage = stg.tile([P, 1024], _DT_BF, tag="stage",
                                             name="stage")
                            nc.vector.tensor_copy(stage[:, :],
                                                  pss[(gh2, bh)][:, :])
                            scr = dsc.tile([P, 1024], _DT_BF, tag="scr",
                                           name="scr")
                            dma_eng = nc.gpsimd if GP_DMA else nc.sync
                            dma_eng.dma_start(out=scr[:, :], in_=stage[:, :])
                            tgt = contribA if k < 2 else contribB
                            cb = k * 2 * G if k < 2 else 0
                            scr_ap = scr[:, :]
                            tgt_ap = tgt[:, :]
                            for bq in range(2):
                                # psum row (br*64+g'), col (bq*512+g*8+bl);
                                # diag g'==g -> contrib[cb+gh*128+br*64+g, b]
                                src = bass.AP(
                                    scr_ap.tensor,
                                    scr_ap.offset + bq * 512,
                                    [[GH * 1024, 2],   # br
                                     [1024 + 8, GH],   # g (row+col step)
                                     [1, 8]])          # bl (contiguous)
                                dst = bass.AP(
                                    tgt_ap.tensor,
                                    tgt_ap.offset
                                    + (cb + gh * 2 * GH) * B
                                    + bh * HB + bq * 8,
                                    [[GH * B, 2],      # br
                                     [B, GH],          # g
                                     [1, 8]])          # bl (contiguous)
                                dma_eng.dma_start(out=dst, in_=src)

                    # ---- AllGathers (gates deferred past k=2 GC) ----
                    if k == 1:
                        gathA = dgg.tile([NCORES * 4 * G, B], _DT_BF,
                                         tag="gA", name="gA",
                                         addr_space="Shared")
                        nc.gpsimd.collective_compute(
                            "AllGather", mybir.AluOpType.bypass,
                            replica_groups=[list(range(NCORES))],
                            ins=[contribA.opt()], outs=[gathA.opt()])
                    elif k == 2:
                        gathB = dgg.tile([NCORES * 2 * G, B], _DT_BF,
                                         tag="gB", name="gB",
                                         addr_space="Shared")
                        nc.gpsimd.collective_compute(
                            "AllGather", mybir.AluOpType.bypass,
                            replica_groups=[list(range(NCORES))],
                            ins=[contribB.opt()], outs=[gathB.opt()])
                    if k + 1 < K:
                        m_tiles = load_mask(k + 1)

                # ---- gathered-chunk gate matmuls ----
                def gate_mms(chunks, gathered, gw_t, kc0, w):
                    for ci, ch in enumerate(chunks):
                        # gathered is [(core, feat), b]; features already on
                        # rows, so chunk loads straight into lhsT layout
                        combT = ctp.tile([P, NCORES * B], _DT_BF,
                                         tag="combT", name="combT")
                        g_ap = gathered[:, :]
                        src = bass.AP(
                            g_ap.tensor, g_ap.offset + ci * G * B,
                            [[B, P],           # feature partition
                             [w * B, NCORES],  # core
                             [1, B]])          # b (contiguous)
                        (nc.gpsimd if GP_DMA else nc.sync).dma_start(
                            out=combT[:, :], in_=src)
                        for c2 in range(NCORES):
                            kc = ch * NCORES + c2
                            nc.tensor.matmul(
                                pgAB[:, :],
                                lhsT=combT[:, c2 * B:(c2 + 1) * B],
                                rhs=gw_t[:, (kc - kc0) * 4 * P:
                                         (kc - kc0 + 1) * 4 * P],
                                start=(kc == 0), stop=(kc == 47))

                pgAB = pgc.tile([B, 4 * G], _DT_F32, tag="gc",
                                name="pgAB")
                gate_mms([0, 1, 2, 3], gathA, gw_a, 0, 4 * G)
                gate_mms([4, 5], gathB, gw_b, 32, 2 * G)
                nc.vector.tensor_add(gacc[:, :], gacc[:, :], pgAB[:, :])

                # ---- LSTM cell ----
                f_t = sml.tile([B, G], _DT_F32, tag="f", name="f")
                i_t = sml.tile([B, G], _DT_F32, tag="i", name="i")
                o_t = sml.tile([B, G], _DT_F32, tag="o", name="o")
                C_t = sml.tile([B, G], _DT_F32, tag="C", name="C")
                inv = 1.0 / GW_SCALE
                nc.scalar.activation(f_t[:, :], gacc[:, 0:G], AF.Sigmoid,
                                     0.0, inv)
                nc.scalar.activation(i_t[:, :], gacc[:, G:2 * G], AF.Sigmoid,
                                     0.0, inv)
                nc.scalar.activation(o_t[:, :], gacc[:, 2 * G:3 * G],
                                     AF.Sigmoid, 0.0, inv)
                nc.scalar.activation(C_t[:, :], gacc[:, 3 * G:4 * G], AF.Tanh,
                                     0.0, inv)
                t1 = sml.tile([B, G], _DT_F32, tag="t1", name="t1")
                nc.vector.tensor_mul(t1[:, :], f_t[:, :], cs_t[:, :])
                t2 = sml.tile([B, G], _DT_F32, tag="t2", name="t2")
                nc.vector.tensor_mul(t2[:, :], i_t[:, :], C_t[:, :])
                hc = sml.tile([B, 2 * G], _DT_F32, tag="hc", name="hc")
                nc.vector.tensor_add(hc[:, G:2 * G], t1[:, :], t2[:, :])
                tc_t = sml.tile([B, G], _DT_F32, tag="tc", name="tcl")
                nc.scalar.activation(tc_t[:, :], hc[:, G:2 * G], AF.Tanh)
                nc.vector.tensor_mul(hc[:, 0:G], o_t[:, :], tc_t[:, :])
                out_ap = p_out[:, :, :]
                dst_out = bass.AP(out_ap.tensor, out_ap.offset,
                                  [[G, B], [B * G, 2], [1, G]])
                nc.sync.dma_start(
                    out=dst_out,
                    in_=hc[:, :].rearrange("b (s g) -> b s g", g=G))

    nc.compile()
    return nc


def _bf(a):
    return np.ascontiguousarray(a.astype(BF16))


def prep_in_maps(input, R1_i, R2_i, R3_i, Hidden_State, Cell_State,
                 GC_R1_w, GC_R2_w, GC_R3_w, GC_NR1_w, GC_NR2_w, GC_NR3_w,
                 fl_w, fl_b, il_w, il_b, ol_w, ol_b, Cl_w, Cl_b):
    """Shard + relayout all inputs for the 8 cores (host side)."""
    input = np.asarray(input, np.float32)
    masks = [np.asarray(m, np.float32) for m in (R1_i, R2_i, R3_i)]
    hs = np.asarray(Hidden_State, np.float32)
    cs = np.asarray(Cell_State, np.float32)
    gcw = [np.asarray(w, np.float32) for w in
           (GC_R1_w, GC_R2_w, GC_R3_w, GC_NR1_w, GC_NR2_w, GC_NR3_w)]
    gates = [(np.asarray(w, np.float32), np.asarray(b, np.float32))
             for w, b in ((fl_w, fl_b), (il_w, il_b), (ol_w, ol_b),
                          (Cl_w, Cl_b))]

    # replicated tensors
    xT = _bf(input.T.reshape(FC, P, B).transpose(1, 0, 2).reshape(P, FC * B))
    hT = hs.T.reshape(FC, P, B).transpose(1, 0, 2).reshape(P, FC * B)
    bias_blk = np.zeros((P, B), np.float32)
    bias_blk[0, :] = 1.0
    hp = _bf(np.concatenate([hT, bias_blk], axis=1))

    # gate-weight feature order: AllGather chunk ch=(k*2+gh) emits
    # rank-major rows (core,b); lhsT partition m=(br*64+g') of that chunk
    # is combined feature (br*3+k)*F + core*G + gh*64 + g'
    feat = np.empty(48 * P, np.int64)
    idx = 0
    for k in range(K):
        for gh in range(2):
            for c2 in range(NCORES):
                for br in range(2):
                    base = (br * 3 + k) * F + c2 * G + gh * GH
                    feat[idx:idx + GH] = np.arange(base, base + GH)
                    idx += GH
    h_feat = np.arange(6 * F, 7 * F)

    in_maps = []
    for c in range(NCORES):
        gsl = slice(c * G, (c + 1) * G)
        # masks: [P, fc, g, b] flattened; m[p, fc*GBB + g*B + b] =
        # mask[b, fc*128+p, c*G+g]
        mks = []
        for k in range(K):
            t = masks[k][:, :, gsl].transpose(1, 2, 0)       # [F, G, B]
            t = (t.reshape(FC, P, G, B).transpose(1, 0, 2, 3)
                 .reshape(P, FC * GBB))
            mks.append(np.ascontiguousarray(
                t.astype(BF16 if MASK_DT[k] == "bf" else FP8)))
        # w-pair stationary tiles: wp[k][p, gh*1024 + fc*128 + br*64 + g']
        wps = []
        for k in range(K):
            wR = gcw[k][:, gsl].reshape(F, 2, GH)            # [F, gh, g']
            wNR = gcw[3 + k][:, gsl].reshape(F, 2, GH)
            blk = np.stack([wR, wNR], axis=2)                # [F, gh, br, g']
            t = (blk.reshape(FC, P, 2, 2, GH).transpose(1, 2, 0, 3, 4)
                 .reshape(P, WPC))                           # [P,(gh,fc,br,g')]
            wps.append(_bf(t))
        cst = np.concatenate([wps[0], xT, hp, wps[1], wps[2]], axis=1)
        assert cst.shape == (P, NCONST)
        # gate weights fp8, packed (kc, gate, m)
        allg = np.empty((4, NKC, P, P), np.float32)          # [gate, kc, kk, m]
        for g_i, (W, bv) in enumerate(gates):
            Wc = W[gsl, :] * GW_SCALE                        # [G(out), 7F]
            gpart = Wc[:, feat].T.reshape(48, P, P)          # [kc, kk, m]
            hpart = Wc[:, h_feat].T.reshape(FC, P, P)
            bias_chunk = np.zeros((1, P, P), np.float32)
            bias_chunk[0, 0, :] = bv[gsl] * GW_SCALE
            allg[g_i] = np.concatenate([gpart, hpart, bias_chunk], axis=0)
        gw = np.ascontiguousarray(
            allg.transpose(2, 1, 0, 3).reshape(P, NKC * 4 * P).astype(FP8))
        in_maps.append({
            "m0": mks[0], "m1": mks[1], "m2": mks[2],
            "cst": np.ascontiguousarray(cst), "gw": gw,
            "cs": np.ascontiguousarray(cs[:, gsl]),
        })
    return in_maps


_cached_nc = None


def _to_np(v):
    try:
        return np.asarray(v)
    except Exception:
        import jax
        return np.asarray(jax.device_put(v, jax.devices("cpu")[0]))


def kernel(**inputs):
    """Full inputs in, full outputs out. Shards across 8 NeuronCores."""
    global _cached_nc
    inputs = {k: _to_np(v) for k, v in inputs.items()}
    # NR1_i/NR2_i/NR3_i are accepted but unused (reference reuses R masks)
    args = {k: inputs[k] for k in (
        "input", "R1_i", "R2_i", "R3_i", "Hidden_State", "Cell_State",
        "GC_R1_w", "GC_R2_w", "GC_R3_w", "GC_NR1_w", "GC_NR2_w", "GC_NR3_w",
        "fl_w", "fl_b", "il_w", "il_b", "ol_w", "ol_b", "Cl_w", "Cl_b")}
    in_maps = prep_in_maps(**args)
    if _cached_nc is None:
        _cached_nc = build_nc(reps=1)
    res = run_bass_kernel_spmd(_cached_nc, in_maps,
                               core_ids=list(range(NCORES)))
    hidden = np.empty((B, F), np.float32)
    cell = np.empty((B, F), np.float32)
    for c in range(NCORES):
        o = res.results[c]["out"]
        hidden[:, c * G:(c + 1) * G] = o[0]
        cell[:, c * G:(c + 1) * G] = o[1]
    return hidden, cell


# revision 25
# speedup vs baseline: 1.1360x; 1.1360x over previous
"""Trainium2 Bass kernel for nn_AdaptiveGraphConvolutionalLSTM.

Reference computation (B=32, F=1024):
    gc_R  = concat_k( einsum('bf,bfg->bg', x, GC_Rk_w  * Rk_i) )   k=1..3
    gc_NR = concat_k( einsum('bf,bfg->bg', x, GC_NRk_w * Rk_i) )   (NR reuses R masks)
    combined = concat([gc_R, gc_NR, hidden])                        # [B, 7F]
    f,i,o = sigmoid(combined @ {fl,il,ol}_w.T + b); C = tanh(combined @ Cl_w.T + Cl_b)
    Cell = f*cell + i*C ; Hidden = o*tanh(Cell)

Distribution over 8 NeuronCores: GC output dim g and gate output columns
sharded (128 columns/core); NR*_i masks never read (unused by reference).

GC structure:
  - v = x (*) m computed ONCE per mask element (VectorE TT, bf16 2x),
    shared by both branches.
  - PE stationary = "w-pair" tile [f-chunk, (br,g')] holding R and NR GC
    weight columns side by side; moving = v. psum out[(br,g'),(g,b)]
    holds both branches' GC results on the g'==g diagonal, extracted via
    one DRAM bounce + strided-AP diagonal DMAs into contrib^T [feat, b].
  - Masks k=1,2 are fp8e4m3 (halves their HBM bytes), converted to bf16
    on ScalarE (most) / VectorE (DVE_CONV set); k=0 stays bf16.
Gate path: AllGather of contrib^T, chunk loads straight into lhsT layout
(no transposes), fp8 gate matmuls (x512 host pre-scale, un-scaled inside
sigmoid/tanh), LSTM cell elementwise.

kernel(**inputs) takes the FULL inputs and returns (Hidden, Cell) full.
"""
import numpy as np
import ml_dtypes

from concourse import bass, bacc, tile, mybir
from concourse.bass_utils import run_bass_kernel_spmd

BF16 = ml_dtypes.bfloat16
FP8 = ml_dtypes.float8_e4m3
GW_SCALE = 512.0
B, F, K, NCORES = 32, 1024, 3, 8
P = 128
FC = F // P      # 8 f-chunks
G = F // NCORES  # 128 g-columns per core
GH = G // 2      # 64
HB = B // 2      # 16
GBB = G * B      # 4096 mask columns per fc block (g-major, b-minor)
NKC = 7 * FC + 1

MASK_DT = ("bf", "f8", "f8")
DVE_CONV = {(2, 5), (2, 6), (2, 7)}

_DT_BF = mybir.dt.bfloat16
_DT_F32 = mybir.dt.float32
_DT_F8 = mybir.dt.float8e4

# bf16 const buffer layout: [wp_k0 | xT | hp | wp_k1 | wp_k2]
WPC = 2 * FC * P                 # 2048
OFF_XT = WPC
OFF_HP = WPC + FC * B            # 2304
OFF_W1 = OFF_HP + 9 * B          # 2592
OFF_W2 = OFF_W1 + WPC            # 4640
NCONST = OFF_W2 + WPC            # 6688
WP_BASE = (0, OFF_W1, OFF_W2)


def build_nc(reps: int = 1):
    nc = bacc.Bacc("TRN2", target_bir_lowering=False, debug=False,
                   num_devices=NCORES)

    p_m = [nc.dram_tensor(f"m{k}", [P, FC * GBB],
                          _DT_BF if MASK_DT[k] == "bf" else _DT_F8,
                          kind="ExternalInput") for k in range(K)]
    p_const = nc.dram_tensor("cst", [P, NCONST], _DT_BF,
                             kind="ExternalInput")
    p_gw = nc.dram_tensor("gw", [P, NKC * 4 * P], _DT_F8,
                          kind="ExternalInput")
    p_cs = nc.dram_tensor("cs", [B, G], _DT_F32, kind="ExternalInput")
    p_out = nc.dram_tensor("out", [2, B, G], _DT_F32, kind="ExternalOutput")

    AF = mybir.ActivationFunctionType
    QGBB = 2 * GBB
    with tile.TileContext(nc) as tc:
        with tc.tile_pool(name="mqb", bufs=3) as mqb, \
             tc.tile_pool(name="mq8", bufs=5) as mq8, \
             tc.tile_pool(name="m16", bufs=2) as m16p, \
             tc.tile_pool(name="vp", bufs=9) as vp, \
             tc.tile_pool(name="cst", bufs=1) as cst, \
             tc.tile_pool(name="stg", bufs=3) as stg, \
             tc.tile_pool(name="ctp", bufs=3) as ctp, \
             tc.tile_pool(name="sml", bufs=2) as sml, \
             tc.tile_pool(name="pgc", bufs=3, space="PSUM") as pgc, \
             tc.tile_pool(name="pgt", bufs=1, space="PSUM") as pgt, \
             tc.tile_pool(name="dsc", bufs=3, space="DRAM") as dsc, \
             tc.tile_pool(name="dcc", bufs=3, space="DRAM") as dcc, \
             tc.tile_pool(name="dgg", bufs=3, space="DRAM") as dgg:

            const_t = cst.tile([P, NCONST], _DT_BF, tag="cst")
            nc.sync.dma_start(out=const_t[:, 0:OFF_W1],
                              in_=p_const[:, 0:OFF_W1])

            def load_mask(k):
                tiles = []
                for q in range(4):
                    if MASK_DT[k] == "bf":
                        mt = mqb.tile([P, QGBB], _DT_BF, tag="mqb",
                                      name="mqb")
                    else:
                        mt = mq8.tile([P, QGBB], _DT_F8, tag="mq8",
                                      name="mq8")
                    nc.sync.dma_start(
                        out=mt[:, :],
                        in_=p_m[k][:, q * QGBB:(q + 1) * QGBB])
                    tiles.append(mt)
                return tiles

            m_tiles0 = load_mask(0)

            nc.sync.dma_start(out=const_t[:, OFF_W1:],
                              in_=p_const[:, OFF_W1:])
            cs_t = cst.tile([B, G], _DT_F32, tag="cs")
            nc.sync.dma_start(out=cs_t[:, :], in_=p_cs[:, :])
            gwh_t = cst.tile([P, 9 * 4 * P], _DT_F8, tag="gwh")
            nc.sync.dma_start(out=gwh_t[:, :], in_=p_gw[:, 48 * 4 * P:])
            gw_a = cst.tile([P, 32 * 4 * P], _DT_F8, tag="gwA")
            nc.sync.dma_start(out=gw_a[:, :], in_=p_gw[:, 0:32 * 4 * P])
            gw_b = cst.tile([P, 16 * 4 * P], _DT_F8, tag="gwB")
            nc.sync.dma_start(out=gw_b[:, :],
                              in_=p_gw[:, 32 * 4 * P:48 * 4 * P])

            def wp_ap(k, gh, fc):
                off = WP_BASE[k] + (gh * FC + fc) * P
                return const_t[:, off:off + P]

            for rep in range(reps):
                m_tiles = m_tiles0 if rep == 0 else load_mask(0)
                pg_t = pgt.tile([B, 4 * G], _DT_F32, tag="pg", name="pg")

                # ---- phase A: hidden-state + bias gate partials ----
                for kc in range(48, 57):
                    lhs = const_t[:, OFF_HP + (kc - 48) * B:
                                  OFF_HP + (kc - 47) * B]
                    nc.tensor.matmul(
                        pg_t[:, :], lhsT=lhs,
                        rhs=gwh_t[:, (kc - 48) * 4 * P:(kc - 47) * 4 * P],
                        start=(kc == 48), stop=False)

                # contrib tensors are [feature, b] (transposed) so the
                # diagonal DMA has contiguous b innermost on both sides
                contribA = dcc.tile([4 * G, B], _DT_BF, tag="cA", name="cA")
                contribB = dcc.tile([2 * G, B], _DT_BF, tag="cB", name="cB")
                gathA = gathB = None

                for k in range(K):
                    # ---- v = x (*) mask, per (fc, gh) tile ----
                    vt = {}
                    for fc in range(FC):
                        msrc = m_tiles[fc // 2][:, (fc % 2) * GBB:
                                                (fc % 2 + 1) * GBB]
                        if MASK_DT[k] == "bf":
                            m16_3 = msrc.rearrange("p (g b) -> p g b", b=B)
                        else:
                            m16 = m16p.tile([P, GBB], _DT_BF, tag="m16",
                                            name="m16")
                            if (k, fc) in DVE_CONV:
                                nc.vector.tensor_copy(m16[:, :], msrc)
                            else:
                                nc.scalar.activation(m16[:, :], msrc,
                                                     AF.Copy)
                            m16_3 = m16[:, :].rearrange("p (g b) -> p g b",
                                                        b=B)
                        x_ap = (const_t[:, OFF_XT + fc * B:
                                        OFF_XT + (fc + 1) * B]
                                .unsqueeze(1).broadcast_to([P, GH, B]))
                        for gh in range(2):
                            v = vp.tile([P, GH * B], _DT_BF, tag="v",
                                        name="v")
                            nc.vector.tensor_mul(
                                v[:, :].rearrange("p (g b) -> p g b", b=B),
                                m16_3[:, gh * GH:(gh + 1) * GH, :], x_ap)
                            vt[(fc, gh)] = v

                    # ---- GC matmuls + per-gh extraction ----
                    for gh in range(2):
                        ps = [pgc.tile([P, 2 * 512], _DT_F32, tag="gc",
                                       name="gcps") for _ in range(2)]
                        for fc in range(FC):
                            lhsT = wp_ap(k, gh, fc)
                            v3 = vt[(fc, gh)][:, :].rearrange(
                                "p (g b) -> p g b", b=B)
                            for bh in range(2):
                                for bq in range(2):
                                    b0 = bh * HB + bq * 8
                                    nc.tensor.matmul(
                                        ps[bh][:, bq * 512:(bq + 1) * 512],
                                        lhsT=lhsT,
                                        rhs=v3[:, :, b0:b0 + 8],
                                        start=(fc == 0),
                                        stop=(fc == FC - 1))
                        for bh in range(2):
                            stage = stg.tile([P, 1024], _DT_BF, tag="stage",
                                             name="stage")
                            nc.vector.tensor_copy(stage[:, :], ps[bh][:, :])
                            scr = dsc.tile([P, 1024], _DT_BF, tag="scr",
                                           name="scr")
                            nc.sync.dma_start(out=scr[:, :], in_=stage[:, :])
                            tgt = contribA if k < 2 else contribB
                            cb = k * 2 * G if k < 2 else 0
                            scr_ap = scr[:, :]
                            tgt_ap = tgt[:, :]
                            for bq in range(2):
                                # psum row (br*64+g'), col (bq*512+g*8+bl);
                                # diag g'==g -> contrib[cb+gh*128+br*64+g, b]
                                src = bass.AP(
                                    scr_ap.tensor,
                                    scr_ap.offset + bq * 512,
                                    [[GH * 1024, 2],   # br
                                     [1024 + 8, GH],   # g (row+col step)
                                     [1, 8]])          # bl (contiguous)
                                dst = bass.AP(
                                    tgt_ap.tensor,
                                    tgt_ap.offset
                                    + (cb + gh * 2 * GH) * B
                                    + bh * HB + bq * 8,
                                    [[GH * B, 2],      # br
                                     [B, GH],          # g
                                     [1, 8]])          # bl (contiguous)
                                nc.sync.dma_start(out=dst, in_=src)

                    # ---- AllGathers (gates deferred past k=2 GC) ----
                    if k == 1:
                        gathA = dgg.tile([NCORES * 4 * G, B], _DT_BF,
                                         tag="gA", name="gA",
                                         addr_space="Shared")
                        nc.gpsimd.collective_compute(
                            "AllGather", mybir.AluOpType.bypass,
                            replica_groups=[list(range(NCORES))],
                            ins=[contribA.opt()], outs=[gathA.opt()])
                    elif k == 2:
                        gathB = dgg.tile([NCORES * 2 * G, B], _DT_BF,
                                         tag="gB", name="gB",
                                         addr_space="Shared")
                        nc.gpsimd.collective_compute(
                            "AllGather", mybir.AluOpType.bypass,
                            replica_groups=[list(range(NCORES))],
                            ins=[contribB.opt()], outs=[gathB.opt()])
                    if k + 1 < K:
                        m_tiles = load_mask(k + 1)

                # ---- gathered-chunk gate matmuls ----
                def gate_mms(chunks, gathered, gw_t, kc0, w):
                    for ci, ch in enumerate(chunks):
                        # gathered is [(core, feat), b]; features already
                        # on rows, so chunks load straight into lhsT layout
                        combT = ctp.tile([P, NCORES * B], _DT_BF,
                                         tag="combT", name="combT")
                        g_ap = gathered[:, :]
                        src = bass.AP(
                            g_ap.tensor, g_ap.offset + ci * G * B,
                            [[B, P],           # feature partition
                             [w * B, NCORES],  # core
                             [1, B]])          # b (contiguous)
                        nc.sync.dma_start(out=combT[:, :], in_=src)
                        for c2 in range(NCORES):
                            kc = ch * NCORES + c2
                            nc.tensor.matmul(
                                pg_t[:, :],
                                lhsT=combT[:, c2 * B:(c2 + 1) * B],
                                rhs=gw_t[:, (kc - kc0) * 4 * P:
                                         (kc - kc0 + 1) * 4 * P],
                                start=False, stop=(kc == 47))

                gate_mms([0, 1, 2, 3], gathA, gw_a, 0, 4 * G)
                gate_mms([4, 5], gathB, gw_b, 32, 2 * G)

                # ---- LSTM cell ----
                f_t = sml.tile([B, G], _DT_F32, tag="f", name="f")
                i_t = sml.tile([B, G], _DT_F32, tag="i", name="i")
                o_t = sml.tile([B, G], _DT_F32, tag="o", name="o")
                C_t = sml.tile([B, G], _DT_F32, tag="C", name="C")
                inv = 1.0 / GW_SCALE
                nc.scalar.activation(f_t[:, :], pg_t[:, 0:G], AF.Sigmoid,
                                     0.0, inv)
                nc.scalar.activation(i_t[:, :], pg_t[:, G:2 * G], AF.Sigmoid,
                                     0.0, inv)
                nc.scalar.activation(o_t[:, :], pg_t[:, 2 * G:3 * G],
                                     AF.Sigmoid, 0.0, inv)
                nc.scalar.activation(C_t[:, :], pg_t[:, 3 * G:4 * G], AF.Tanh,
                                     0.0, inv)
                t1 = sml.tile([B, G], _DT_F32, tag="t1", name="t1")
                nc.vector.tensor_mul(t1[:, :], f_t[:, :], cs_t[:, :])
                t2 = sml.tile([B, G], _DT_F32, tag="t2", name="t2")
                nc.vector.tensor_mul(t2[:, :], i_t[:, :], C_t[:, :])
                hc = sml.tile([B, 2 * G], _DT_F32, tag="hc", name="hc")
                nc.vector.tensor_add(hc[:, G:2 * G], t1[:, :], t2[:, :])
                tc_t = sml.tile([B, G], _DT_F32, tag="tc", name="tcl")
                nc.scalar.activation(tc_t[:, :], hc[:, G:2 * G], AF.Tanh)
                nc.vector.tensor_mul(hc[:, 0:G], o_t[:, :], tc_t[:, :])
                out_ap = p_out[:, :, :]
                dst_out = bass.AP(out_ap.tensor, out_ap.offset,
                                  [[G, B], [B * G, 2], [1, G]])
                nc.sync.dma_start(
                    out=dst_out,
                    in_=hc[:, :].rearrange("b (s g) -> b s g", g=G))

    nc.compile()
    return nc


def _bf(a):
    return np.ascontiguousarray(a.astype(BF16))


def prep_in_maps(input, R1_i, R2_i, R3_i, Hidden_State, Cell_State,
                 GC_R1_w, GC_R2_w, GC_R3_w, GC_NR1_w, GC_NR2_w, GC_NR3_w,
                 fl_w, fl_b, il_w, il_b, ol_w, ol_b, Cl_w, Cl_b):
    """Shard + relayout all inputs for the 8 cores (host side)."""
    input = np.asarray(input, np.float32)
    masks = [np.asarray(m, np.float32) for m in (R1_i, R2_i, R3_i)]
    hs = np.asarray(Hidden_State, np.float32)
    cs = np.asarray(Cell_State, np.float32)
    gcw = [np.asarray(w, np.float32) for w in
           (GC_R1_w, GC_R2_w, GC_R3_w, GC_NR1_w, GC_NR2_w, GC_NR3_w)]
    gates = [(np.asarray(w, np.float32), np.asarray(b, np.float32))
             for w, b in ((fl_w, fl_b), (il_w, il_b), (ol_w, ol_b),
                          (Cl_w, Cl_b))]

    xT = _bf(input.T.reshape(FC, P, B).transpose(1, 0, 2).reshape(P, FC * B))
    hT = hs.T.reshape(FC, P, B).transpose(1, 0, 2).reshape(P, FC * B)
    bias_blk = np.zeros((P, B), np.float32)
    bias_blk[0, :] = 1.0
    hp = _bf(np.concatenate([hT, bias_blk], axis=1))

    # gate-weight feature order: AllGather chunk ch=(k*2+gh) emits
    # rank-major rows (core, feat); lhsT partition m=(br*64+g') of chunk
    # ch is combined feature (br*3+k)*F + core*G + gh*64 + g'
    feat = np.empty(48 * P, np.int64)
    idx = 0
    for k in range(K):
        for gh in range(2):
            for c2 in range(NCORES):
                for br in range(2):
                    base = (br * 3 + k) * F + c2 * G + gh * GH
                    feat[idx:idx + GH] = np.arange(base, base + GH)
                    idx += GH
    h_feat = np.arange(6 * F, 7 * F)

    in_maps = []
    for c in range(NCORES):
        gsl = slice(c * G, (c + 1) * G)
        mks = []
        for k in range(K):
            t = masks[k][:, :, gsl].transpose(1, 2, 0)       # [F, G, B]
            t = (t.reshape(FC, P, G, B).transpose(1, 0, 2, 3)
                 .reshape(P, FC * GBB))
            mks.append(np.ascontiguousarray(
                t.astype(BF16 if MASK_DT[k] == "bf" else FP8)))
        wps = []
        for k in range(K):
            wR = gcw[k][:, gsl].reshape(F, 2, GH)
            wNR = gcw[3 + k][:, gsl].reshape(F, 2, GH)
            blk = np.stack([wR, wNR], axis=2)                # [F, gh, br, g']
            t = (blk.reshape(FC, P, 2, 2, GH).transpose(1, 2, 0, 3, 4)
                 .reshape(P, WPC))
            wps.append(_bf(t))
        cst = np.concatenate([wps[0], xT, hp, wps[1], wps[2]], axis=1)
        assert cst.shape == (P, NCONST)
        allg = np.empty((4, NKC, P, P), np.float32)
        for g_i, (W, bv) in enumerate(gates):
            Wc = W[gsl, :] * GW_SCALE
            gpart = Wc[:, feat].T.reshape(48, P, P)
            hpart = Wc[:, h_feat].T.reshape(FC, P, P)
            bias_chunk = np.zeros((1, P, P), np.float32)
            bias_chunk[0, 0, :] = bv[gsl] * GW_SCALE
            allg[g_i] = np.concatenate([gpart, hpart, bias_chunk], axis=0)
        gw = np.ascontiguousarray(
            allg.transpose(2, 1, 0, 3).reshape(P, NKC * 4 * P).astype(FP8))
        in_maps.append({
            "m0": mks[0], "m1": mks[1], "m2": mks[2],
            "cst": np.ascontiguousarray(cst), "gw": gw,
            "cs": np.ascontiguousarray(cs[:, gsl]),
        })
    return in_maps


_cached_nc = None


def _to_np(v):
    try:
        return np.asarray(v)
    except Exception:
        import jax
        return np.asarray(jax.device_put(v, jax.devices("cpu")[0]))


def kernel(**inputs):
    """Full inputs in, full outputs out. Shards across 8 NeuronCores."""
    global _cached_nc
    inputs = {k: _to_np(v) for k, v in inputs.items()}
    args = {k: inputs[k] for k in (
        "input", "R1_i", "R2_i", "R3_i", "Hidden_State", "Cell_State",
        "GC_R1_w", "GC_R2_w", "GC_R3_w", "GC_NR1_w", "GC_NR2_w", "GC_NR3_w",
        "fl_w", "fl_b", "il_w", "il_b", "ol_w", "ol_b", "Cl_w", "Cl_b")}
    in_maps = prep_in_maps(**args)
    if _cached_nc is None:
        _cached_nc = build_nc(reps=1)
    res = run_bass_kernel_spmd(_cached_nc, in_maps,
                               core_ids=list(range(NCORES)))
    hidden = np.empty((B, F), np.float32)
    cell = np.empty((B, F), np.float32)
    for c in range(NCORES):
        o = res.results[c]["out"]
        hidden[:, c * G:(c + 1) * G] = o[0]
        cell[:, c * G:(c + 1) * G] = o[1]
    return hidden, cell
